# revision 1
# baseline (speedup 1.0000x reference)
"""CrossAttention TRN2 kernel — context-parallel over (batch, seq-chunk), all-bf16, no collectives.

8 cores: core c -> batch b=c//4, seq chunk j=c%4 (512 query rows).
Per core (all matmul inputs bf16, PSUM accumulation fp32):
  A. load full ctx_b (cast->bf16 in DMA), PE-transpose -> ctxT; kT = Wk.T@ctxT (full M),
     vaug = ctx@Wv ones-augmented (full M)
  B. load x chunk, transpose -> xT; qT = Wq.T@xT
  C. flash attention, S.T orientation: S.T[m,n] = kT_h-slices.T @ qT_h (head pairs packed in K-row groups)
     exp on ACT from 3-bank psum groups -> bf16 SBUF; AV: oT_h[65,n] += vaug_h.T @ expST (row 64 = denom)
     normalize via DVE reciprocal + gpsimd partition_broadcast -> oT_sb bf16
  D. out[n,1024] = oT.T @ Wo + ones-row x bo bias; write [512,1024] fp32
PSUM (8 banks): phase A/B: ptr 2 + pproj 2; phase C: sA 3 + sB 3 + oA 1 + oB 1; phase D: pout 2.
"""
import sys
sys.path.insert(0, '/opt/trn_rl_repo')
import numpy as np
import concourse.bass as bass
import concourse.mybir as mybir
import concourse.tile as tile
from concourse import bacc
from concourse.masks import make_identity

F32 = mybir.dt.float32
BF16 = mybir.dt.bfloat16
AF = mybir.ActivationFunctionType

B, N, M, KDIM, H, D = 2, 2048, 2048, 1024, 8, 64
INNER = H * D          # 512
NC = 512               # query rows per core chunk
SCALE = D ** -0.5      # 0.125
KC = KDIM // 128       # 8 k-chunks
DC = INNER // 128      # 4 inner chunks (= head pairs)
NT = NC // 128         # 4 n-tiles per core
MC = M // 128          # 16 m-chunks
MG = M // 512          # 4 m-groups of 512
VW = 2 * (D + 1)       # 130: [vA(64) | 1 | vB(64) | 1] per head pair
GRP = 3                # m-chunks per exp group


def build_kernel():
    nc = bacc.Bacc("TRN2", target_bir_lowering=False, debug=False, num_devices=8)
    X = nc.dram_tensor("xc", [NC, KDIM], F32, kind="ExternalInput")
    CTX = nc.dram_tensor("ctxc", [M, KDIM], F32, kind="ExternalInput")
    WQ = nc.dram_tensor("Wq", [KDIM, INNER], F32, kind="ExternalInput")
    WK = nc.dram_tensor("Wk", [KDIM, INNER], F32, kind="ExternalInput")
    WV = nc.dram_tensor("Wv", [KDIM, INNER], F32, kind="ExternalInput")
    WO = nc.dram_tensor("Wo", [INNER, KDIM], F32, kind="ExternalInput")
    BO = nc.dram_tensor("bo", [1, KDIM], F32, kind="ExternalInput")
    OUT = nc.dram_tensor("outc", [NC, KDIM], F32, kind="ExternalOutput")

    with tile.TileContext(nc) as tc:
        import contextlib
        with contextlib.ExitStack() as ctx:
            sb = ctx.enter_context(tc.tile_pool(name="sb", bufs=1))
            stage = ctx.enter_context(tc.tile_pool(name="stage", bufs=3))

            ident = sb.tile([128, 128], BF16, tag="ident")
            make_identity(nc, ident[:])

            def load_w(pool, wdram, name, rows, cols):
                out = []
                for k in range(rows // 128):
                    wr = pool.tile([128, cols], BF16, tag=f"{name}{k}", name=f"{name}{k}")
                    nc.gpsimd.dma_start(wr[:], wdram[128 * k:128 * (k + 1), :])
                    out.append(wr)
                return out

            def load_transpose(pool, pst, src_dram, rows, name):
                # -> [128, KC, rows] bf16 view; chunk k = [:, k, :]
                rt = rows // 128
                tT = pool.tile([128, KC * rows], BF16, tag=f"{name}T", name=f"{name}T")
                tT3 = tT[:].rearrange("p (k n) -> p k n", k=KC)
                for t in range(rt):
                    nat = stage.tile([128, KDIM], BF16, tag="nat")
                    nc.gpsimd.dma_start(nat[:], src_dram[128 * t:128 * (t + 1), :])
                    for kg in range(KC // 4):
                        p = pst.tile([128, 512], BF16, tag="ptr")
                        for i in range(4):
                            k = 4 * kg + i
                            nc.tensor.transpose(p[:, 128 * i:128 * (i + 1)],
                                                nat[:, 128 * k:128 * (k + 1)], ident[:])
                        dst = tT3[:, 4 * kg:4 * (kg + 1), 128 * t:128 * (t + 1)]
                        src = p[:].rearrange("p (i c) -> p i c", i=4)
                        nc.vector.tensor_copy(dst, src)
                return tT3

            # ---------- phase A: full ctx -> ctxT -> kT, vaug ----------
            kT = [sb.tile([128, M], BF16, tag=f"kT{dc}", name=f"kT{dc}") for dc in range(DC)]
            vaug = [sb.tile([128, VW * DC], BF16, tag=f"vg{mt}", name=f"vg{mt}")
                    for mt in range(MC)]
            with (tc.tile_pool(name="pA", bufs=1) as pA,
                  tc.tile_pool(name="pAps", bufs=2, space="PSUM") as pAps):
                ctxT = load_transpose(pA, pAps, CTX, M, "ctx")
                wk = load_w(pA, WK, "wk", KDIM, INNER)
                wv = load_w(pA, WV, "wv", KDIM, INNER)

                for dc in range(DC):
                    for mg in range(MG):
                        p = pAps.tile([128, 512], F32, tag="pproj")
                        for k in range(KC):
                            nc.tensor.matmul(p[:], wk[k][:, 128 * dc:128 * (dc + 1)],
                                             ctxT[:, k, 512 * mg:512 * (mg + 1)],
                                             start=(k == 0), stop=(k == KC - 1))
                        nc.vector.tensor_copy(kT[dc][:, 512 * mg:512 * (mg + 1)], p[:])

                for mt in range(MC):
                    p = pAps.tile([128, 512], F32, tag="pproj")
                    for k in range(KC):
                        nc.tensor.matmul(p[:], ctxT[:, k, 128 * mt:128 * (mt + 1)], wv[k][:],
                                         start=(k == 0), stop=(k == KC - 1))
                    t = vaug[mt]
                    pv = p[:].rearrange("p (hp two d) -> p hp two d", hp=DC, two=2)
                    tv = t[:].rearrange("p (hp w) -> p hp w", hp=DC)[:, :, 0:VW].rearrange(
                        "p hp (two dd) -> p hp two dd", two=2)[:, :, :, 0:D]
                    nc.vector.tensor_copy(tv, pv)
                    ones = t[:].rearrange("p (hp w) -> p hp w", hp=DC).rearrange(
                        "p hp (two dd) -> p hp two dd", two=2)[:, :, :, D:D + 1]
                    nc.vector.memset(ones, 1.0)

            # ---------- phase B: x chunk -> xT -> qT ----------
            qT = [sb.tile([128, NC], BF16, tag=f"qT{dc}", name=f"qT{dc}") for dc in range(DC)]
            with (tc.tile_pool(name="pB", bufs=1) as pB,
                  tc.tile_pool(name="pBps", bufs=2, space="PSUM") as pBps):
                xT = load_transpose(pB, pBps, X, NC, "x")
                wq = load_w(pB, WQ, "wq", KDIM, INNER)
                for dc in range(DC):
                    p = pBps.tile([128, NC], F32, tag="pproj")
                    for k in range(KC):
                        nc.tensor.matmul(p[:], wq[k][:, 128 * dc:128 * (dc + 1)],
                                         xT[:, k, :], start=(k == 0), stop=(k == KC - 1))
                    nc.vector.tensor_copy(qT[dc][:], p[:])

            wo = load_w(sb, WO, "wo", INNER, KDIM)
            bo_r = sb.tile([1, KDIM], BF16, tag="bo_r")
            nc.gpsimd.dma_start(bo_r[:], BO[:])
            ones_row = sb.tile([1, 128], BF16, tag="ones_row")
            nc.vector.memset(ones_row[:], 1.0)

            # ---------- phase C: attention (head-sequential, 6-chunk exp groups) ----------
            GRP6 = 6
            oT_sb = []
            with (tc.tile_pool(name="psS", bufs=1, space="PSUM") as psS,
                  tc.tile_pool(name="psO", bufs=2, space="PSUM") as psO):
                for hp in range(DC):
                    o = sb.tile([128, NC], BF16, tag=f"oT{hp}", name=f"oT{hp}")
                    for head in range(2):
                        base_k = 64 * head
                        vbase = VW * hp + (D + 1) * head
                        oX = psO.tile([D + 1, NC], F32, tag="oX")

                        def do_av(pend):
                            mcs_p, e_p = pend
                            for i, mc in enumerate(mcs_p):
                                v = vaug[mc][:, vbase:vbase + D + 1]
                                nc.tensor.matmul(oX[:], v, e_p[:, 512 * i:512 * (i + 1)],
                                                 start=(mc == 0), stop=(mc == MC - 1))

                        pending = None
                        for s0 in range(0, MC, GRP6):
                            mcs = list(range(s0, min(s0 + GRP6, MC)))
                            w = 512 * len(mcs)
                            s = psS.tile([128, 512 * GRP6], F32, tag="s")
                            for i, mc in enumerate(mcs):
                                ksl = kT[hp][base_k:base_k + 64, 128 * mc:128 * (mc + 1)]
                                nc.tensor.matmul(s[:, 512 * i:512 * (i + 1)], ksl,
                                                 qT[hp][base_k:base_k + 64, :],
                                                 start=True, stop=True)
                            if pending is not None:
                                do_av(pending)
                            e = stage.tile([128, 512 * GRP6], BF16, tag="e")
                            nc.scalar.activation(e[:, 0:w], s[:, 0:w], AF.Exp,
                                                 bias=0.0, scale=SCALE)
                            pending = (mcs, e)
                        do_av(pending)
                        rec = stage.tile([1, NC], F32, tag="rec")
                        nc.vector.reciprocal(rec[:], oX[D:D + 1, :])
                        rec_b = stage.tile([D, NC], F32, tag="rec_b")
                        nc.gpsimd.partition_broadcast(rec_b[:], rec[:])
                        nc.vector.tensor_mul(o[base_k:base_k + D, :], oX[0:D, :], rec_b[:])
                    oT_sb.append(o)

            # ---------- phase D: O projection + bias ----------
            with tc.tile_pool(name="psout", bufs=2, space="PSUM") as psout:
                for nt in range(NT):
                    for hf in range(2):
                        p = psout.tile([128, 512], F32, tag="pout")
                        for ic in range(DC):
                            nc.tensor.matmul(p[:], oT_sb[ic][:, 128 * nt:128 * (nt + 1)],
                                             wo[ic][:, 512 * hf:512 * (hf + 1)],
                                             start=(ic == 0), stop=False)
                        nc.tensor.matmul(p[:], ones_row[:], bo_r[:, 512 * hf:512 * (hf + 1)],
                                         start=False, stop=True)
                        osb = stage.tile([128, 512], F32, tag="osb")
                        nc.vector.tensor_copy(osb[:], p[:])
                        nc.sync.dma_start(
                            OUT[128 * nt:128 * (nt + 1), 512 * hf:512 * (hf + 1)], osb[:])
    nc.compile()
    return nc


def shard_inputs(inputs):
    """full inputs dict -> list of 8 per-core in_maps"""
    x, ctx = np.asarray(inputs["x"]), np.asarray(inputs["context"])
    maps = []
    for c in range(8):
        b, j = c // 4, c % 4
        maps.append({
            "xc": np.ascontiguousarray(x[b, NC * j:NC * (j + 1), :]),
            "ctxc": np.ascontiguousarray(ctx[b]),
            "Wq": np.asarray(inputs["Wq"]), "Wk": np.asarray(inputs["Wk"]),
            "Wv": np.asarray(inputs["Wv"]), "Wo": np.asarray(inputs["Wo"]),
            "bo": np.asarray(inputs["bo"]).reshape(1, KDIM),
        })
    return maps


def unshard_outputs(results):
    out = np.empty((B, N, KDIM), dtype=np.float32)
    for c in range(8):
        b, j = c // 4, c % 4
        out[b, NC * j:NC * (j + 1), :] = results[c]["outc"]
    return out


_CACHED = {}


def kernel(**inputs):
    """Full unsharded inputs -> full output [2, 2048, 1024] fp32. Runs on 8 NeuronCores."""
    from concourse.bass_utils import run_bass_kernel_spmd
    if "nc" not in _CACHED:
        _CACHED["nc"] = build_kernel()
    nc = _CACHED["nc"]
    maps = shard_inputs(inputs)
    res = run_bass_kernel_spmd(nc, maps, list(range(8)))
    return unshard_outputs(res.results)


_CACHED = {}


def kernel(**inputs):
    """Full unsharded inputs -> full output [2, 2048, 1024] fp32. Runs on 8 NeuronCores."""
    from concourse.bass_utils import run_bass_kernel_spmd
    if "nc" not in _CACHED:
        _CACHED["nc"] = build_kernel()
    nc = _CACHED["nc"]
    maps = shard_inputs(inputs)
    res = run_bass_kernel_spmd(nc, maps, list(range(8)))
    return unshard_outputs(res.results)



# revision 5
# speedup vs baseline: 1.0278x; 1.0278x over previous
"""CrossAttention TRN2 kernel — context-parallel over (batch, seq-chunk), all-bf16, no collectives.

8 cores: core c -> batch b=c//4, seq chunk j=c%4 (512 query rows).
v2: PE kept continuously busy (p-state), transposes moved off PE onto the DMA
xbar (dma_start_transpose), proj PSUM->SBUF copies on ACT (scalar.copy),
softmax reciprocal via DVE reciprocal_approx_fast (batched off the ACT table),
output bias via DVE add, batched 3D DMA loads issued once on gpsimd.

Per core (all matmul inputs bf16, PSUM accumulation fp32):
  load:  x chunk + full ctx_b + W* as bf16 via gpsimd casting DMAs (3D APs)
  xbar:  ctxT[p,kc,m], xT[p,kc,n] via sync dma_start_transpose
  proj:  qT = Wq.T@xT; kT = Wk.T@ctxT; vaug = ctx@Wv (ones-augmented)
  attn:  S.T[m,n] = kT_h.T @ qT_h in 2-chunk PSUM groups; exp on ACT -> bf16
         SBUF; AV: oX[65,n] += vaug_h.T @ expST (row 64 = denom);
         normalize: DVE recip_approx + gpsimd partition_broadcast + DVE mul
  out:   out[n,1024] = oT.T @ Wo (+bias via DVE add) -> fp32 DMA out
PSUM: proj 4 banks; attn: S 3x2 banks + oX 2 banks; out: 2x2 banks.
"""
import sys
sys.path.insert(0, '/opt/trn_rl_repo')
import contextlib
import numpy as np
import concourse.bass as bass
import concourse.mybir as mybir
import concourse.tile as tile
from concourse import bacc

F32 = mybir.dt.float32
BF16 = mybir.dt.bfloat16
AF = mybir.ActivationFunctionType

B, N, M, KDIM, H, D = 2, 2048, 2048, 1024, 8, 64
INNER = H * D          # 512
NC = 512               # query rows per core chunk
SCALE = D ** -0.5      # 0.125
KC = KDIM // 128       # 8 k-chunks
DC = INNER // 128      # 4 inner chunks (= head pairs)
NT = NC // 128         # 4 n-tiles per core
MC = M // 128          # 16 m-chunks
MG = M // 512          # 4 m-groups of 512
VW = 2 * (D + 1)       # 130: [vA(64) | 1 | vB(64) | 1] per head pair
GRP = 2                # m-chunks per exp group


def build_kernel():
    nc = bacc.Bacc("TRN2", target_bir_lowering=False, debug=False, num_devices=8)
    X = nc.dram_tensor("xc", [NC, KDIM], F32, kind="ExternalInput")
    CTX = nc.dram_tensor("ctxc", [M, KDIM], F32, kind="ExternalInput")
    WQ = nc.dram_tensor("Wq", [KDIM, INNER], F32, kind="ExternalInput")
    WK = nc.dram_tensor("Wk", [KDIM, INNER], F32, kind="ExternalInput")
    WV = nc.dram_tensor("Wv", [KDIM, INNER], F32, kind="ExternalInput")
    WO = nc.dram_tensor("Wo", [INNER, KDIM], F32, kind="ExternalInput")
    BO = nc.dram_tensor("bo", [1, KDIM], F32, kind="ExternalInput")
    OUT = nc.dram_tensor("outc", [NC, KDIM], F32, kind="ExternalOutput")

    with tile.TileContext(nc) as tc:
        with contextlib.ExitStack() as ctx:
            sb = ctx.enter_context(tc.tile_pool(name="sb", bufs=1))
            stage = ctx.enter_context(tc.tile_pool(name="stage", bufs=3))

            # ---------- persistent SBUF tiles ----------
            xn = sb.tile([128, NT * KDIM], BF16, tag="xn")
            xn3 = xn[:].rearrange("p (t k) -> p t k", t=NT)
            ctxn = [sb.tile([128, 4 * KDIM], BF16, tag=f"ctxn{g}", name=f"ctxn{g}") for g in range(4)]
            wq = sb.tile([128, KC * INNER], BF16, tag="wq")
            wk = sb.tile([128, KC * INNER], BF16, tag="wk")
            wv = sb.tile([128, KC * INNER], BF16, tag="wv")
            wo = sb.tile([128, DC * KDIM], BF16, tag="wo")
            wq3 = wq[:].rearrange("p (k d) -> p k d", k=KC)
            wk3 = wk[:].rearrange("p (k d) -> p k d", k=KC)
            wv3 = wv[:].rearrange("p (k d) -> p k d", k=KC)
            wo3 = wo[:].rearrange("p (i o) -> p i o", i=DC)
            ctxT = sb.tile([128, KC * M], BF16, tag="ctxT")
            ctxT3 = ctxT[:].rearrange("p (k m) -> p k m", k=KC)
            xT = sb.tile([128, KC * NC], BF16, tag="xT")
            xT3 = xT[:].rearrange("p (k n) -> p k n", k=KC)
            kT = [sb.tile([128, M], BF16, tag=f"kT{dc}", name=f"kT{dc}") for dc in range(DC)]
            vaug = [sb.tile([128, VW * DC], BF16, tag=f"vg{mt}", name=f"vg{mt}") for mt in range(MC)]
            qT = [sb.tile([128, NC], BF16, tag=f"qT{dc}", name=f"qT{dc}") for dc in range(DC)]
            oT = [sb.tile([128, NC], BF16, tag=f"oT{hp}", name=f"oT{hp}") for hp in range(DC)]
            bo_r = sb.tile([1, KDIM], F32, tag="bo_r")
            bias_bc = sb.tile([128, KDIM], F32, tag="bias_bc")

            # ---------- input DMAs (gpsimd = casting swdge), priority order ----------
            nc.gpsimd.dma_start(xn3, X[:].rearrange("(t p) k -> p t k", p=128))
            nc.gpsimd.dma_start(wq3, WQ[:].rearrange("(k p) d -> p k d", p=128))
            nc.gpsimd.dma_start(wk3, WK[:].rearrange("(k p) d -> p k d", p=128))
            nc.gpsimd.dma_start(
                ctxn[0][:].rearrange("p (t k) -> p t k", t=4),
                CTX[0:512, :].rearrange("(t p) k -> p t k", p=128))
            nc.gpsimd.dma_start(wv3, WV[:].rearrange("(k p) d -> p k d", p=128))
            for g in range(1, 4):
                nc.gpsimd.dma_start(
                    ctxn[g][:].rearrange("p (t k) -> p t k", t=4),
                    CTX[512 * g:512 * (g + 1), :].rearrange("(t p) k -> p t k", p=128))
            nc.gpsimd.dma_start(wo3, WO[:].rearrange("(i p) o -> p i o", p=128))
            nc.sync.dma_start(bo_r[:], BO[:])
            nc.gpsimd.partition_broadcast(bias_bc[:], bo_r[:])

            # ones columns of vaug (constant, written once)
            for mt in range(MC):
                ones = vaug[mt][:].rearrange("p (hp w) -> p hp w", hp=DC).rearrange(
                    "p hp (two dd) -> p hp two dd", two=2)[:, :, :, D:D + 1]
                nc.vector.memset(ones, 1.0)

            # ---------- xbar transposes (sync hwdge) ----------
            for t in range(NT):
                nc.sync.dma_start_transpose(xT3[:, :, 128 * t:128 * (t + 1)], xn3[:, t, :])
            for g in range(4):
                cg = ctxn[g][:].rearrange("p (t k) -> p t k", t=4)
                for t in range(4):
                    mt = 4 * g + t
                    nc.sync.dma_start_transpose(
                        ctxT3[:, :, 128 * mt:128 * (mt + 1)], cg[:, t, :])

            # ---------- projections (PE), PSUM->SBUF copies on ACT ----------
            with tc.tile_pool(name="pp", bufs=4, space="PSUM") as pp:
                for dc in range(DC):
                    p = pp.tile([128, NC], F32, tag="pp")
                    for k in range(KC):
                        nc.tensor.matmul(p[:], wq3[:, k, 128 * dc:128 * (dc + 1)],
                                         xT3[:, k, :], start=(k == 0), stop=(k == KC - 1))
                    nc.scalar.copy(qT[dc][:], p[:])

                for g in range(4):
                    for dc in range(DC):
                        p = pp.tile([128, 512], F32, tag="pp")
                        for k in range(KC):
                            nc.tensor.matmul(p[:], wk3[:, k, 128 * dc:128 * (dc + 1)],
                                             ctxT3[:, k, 512 * g:512 * (g + 1)],
                                             start=(k == 0), stop=(k == KC - 1))
                        nc.scalar.copy(kT[dc][:, 512 * g:512 * (g + 1)], p[:])
                    for t in range(4):
                        mt = 4 * g + t
                        p = pp.tile([128, 512], F32, tag="pp")
                        for k in range(KC):
                            nc.tensor.matmul(p[:], ctxT3[:, k, 128 * mt:128 * (mt + 1)],
                                             wv3[:, k, :], start=(k == 0), stop=(k == KC - 1))
                        pv = p[:].rearrange("p (hp two d) -> p hp two d", hp=DC, two=2)
                        tv = vaug[mt][:].rearrange("p (hp w) -> p hp w", hp=DC)[
                            :, :, 0:VW].rearrange(
                            "p hp (two dd) -> p hp two dd", two=2)[:, :, :, 0:D]
                        nc.vector.tensor_copy(tv, pv)

            # ---------- attention ----------
            NG = MC // GRP  # 8 groups of 2 m-chunks
            with (tc.tile_pool(name="psS", bufs=3, space="PSUM") as psS,
                  tc.tile_pool(name="psO", bufs=2, space="PSUM") as psO,
                  tc.tile_pool(name="se", bufs=3) as se):
                for hp in range(DC):
                    for head in range(2):
                        bk = 64 * head
                        vb = VW * hp + (D + 1) * head
                        oX = psO.tile([128, NC], F32, tag="oX")

                        def do_av(pend):
                            g_p, e_p = pend
                            for i in range(GRP):
                                mc = GRP * g_p + i
                                v = vaug[mc][:, vb:vb + D + 1]
                                nc.tensor.matmul(oX[0:D + 1, :], v,
                                                 e_p[:, 512 * i:512 * (i + 1)],
                                                 start=(mc == 0), stop=(mc == MC - 1))

                        pending = None
                        for g in range(NG):
                            s = psS.tile([128, 512 * GRP], F32, tag="s")
                            for i in range(GRP):
                                mc = GRP * g + i
                                ksl = kT[hp][bk:bk + 64, 128 * mc:128 * (mc + 1)]
                                nc.tensor.matmul(s[:, 512 * i:512 * (i + 1)], ksl,
                                                 qT[hp][bk:bk + 64, :],
                                                 start=True, stop=True)
                            if pending is not None:
                                do_av(pending)
                            e = se.tile([128, 512 * GRP], BF16, tag="e")
                            nc.scalar.activation(e[:], s[:], AF.Exp, bias=0.0, scale=SCALE)
                            pending = (g, e)
                        do_av(pending)
                        den = stage.tile([1, NC], F32, tag="den")
                        nc.scalar.copy(den[:], oX[D:D + 1, :])
                        rec = stage.tile([1, NC], F32, tag="rec")
                        nc.vector.reciprocal_approx_fast(rec[:], den[:])
                        rec_b = stage.tile([D, NC], F32, tag="rec_b")
                        nc.gpsimd.partition_broadcast(rec_b[:], rec[:])
                        nc.vector.tensor_mul(oT[hp][bk:bk + D, :], oX[0:D, :], rec_b[:])

            # ---------- O projection + bias ----------
            with (tc.tile_pool(name="psD", bufs=2, space="PSUM") as psD,
                  tc.tile_pool(name="so", bufs=2) as so):
                for nt in range(NT):
                    for hf in range(2):
                        p = psD.tile([128, 512], F32, tag="pout")
                        for ic in range(DC):
                            nc.tensor.matmul(p[:], oT[ic][:, 128 * nt:128 * (nt + 1)],
                                             wo3[:, ic, 512 * hf:512 * (hf + 1)],
                                             start=(ic == 0), stop=(ic == DC - 1))
                        osb = so.tile([128, 512], F32, tag="osb")
                        nc.vector.tensor_add(osb[:], p[:], bias_bc[:, 512 * hf:512 * (hf + 1)])
                        nc.sync.dma_start(
                            OUT[128 * nt:128 * (nt + 1), 512 * hf:512 * (hf + 1)], osb[:])
    nc.compile()
    return nc


def shard_inputs(inputs):
    """full inputs dict -> list of 8 per-core in_maps"""
    x, ctx = np.asarray(inputs["x"]), np.asarray(inputs["context"])
    maps = []
    for c in range(8):
        b, j = c // 4, c % 4
        maps.append({
            "xc": np.ascontiguousarray(x[b, NC * j:NC * (j + 1), :]),
            "ctxc": np.ascontiguousarray(ctx[b]),
            "Wq": np.asarray(inputs["Wq"]), "Wk": np.asarray(inputs["Wk"]),
            "Wv": np.asarray(inputs["Wv"]), "Wo": np.asarray(inputs["Wo"]),
            "bo": np.asarray(inputs["bo"]).reshape(1, KDIM),
        })
    return maps


def unshard_outputs(results):
    out = np.empty((B, N, KDIM), dtype=np.float32)
    for c in range(8):
        b, j = c // 4, c % 4
        out[b, NC * j:NC * (j + 1), :] = results[c]["outc"]
    return out


_CACHED = {}


def kernel(**inputs):
    """Full unsharded inputs -> full output [2, 2048, 1024] fp32. Runs on 8 NeuronCores."""
    from concourse.bass_utils import run_bass_kernel_spmd
    if "nc" not in _CACHED:
        _CACHED["nc"] = build_kernel()
    nc = _CACHED["nc"]
    maps = shard_inputs(inputs)
    res = run_bass_kernel_spmd(nc, maps, list(range(8)))
    return unshard_outputs(res.results)
